# revision 16
# baseline (speedup 1.0000x reference)
"""Trainium2 Bass kernel for the ConvModule problem.

Computes, for x (B=16, T=1024, C=512) fp32:
    h = LayerNorm_C(x) -> pw conv C->2C + Swish -> k=5 conv 2C->2C
      -> GLU -> BatchNorm(eval) -> pw conv C->C
Data-parallel over batch across 8 NeuronCores (2 batches/core, weights
replicated).  LN gamma/beta folded into w1/b1 and BN folded into w3/b3 on
the host, so the device only does: normalize, three matmul stages, Swish,
GLU.

Device data layout is [channel, time] (channels on partitions) for the
whole matmul chain; the final conv swaps matmul operands (activations as
the stationary lhsT) so its PSUM output lands directly in [time, channel]
DRAM layout.
"""

import os
from contextlib import ExitStack

import numpy as np

import concourse.bass as bass
import concourse.bacc as bacc
import concourse.tile as tile
from concourse import mybir
from concourse.masks import make_identity
from concourse.bass_utils import run_bass_kernel_spmd

B, T, C, K = 16, 1024, 512, 5
EPS_LN = 1e-5
EPS_BN = 1e-5
NCORES = 8
BLOC = B // NCORES          # batches per core
P = 128                     # SBUF partitions
CB = C // P                 # 4 channel blocks of the C dim
OB = (2 * C) // P           # 8 channel blocks of the 2C dim
TH = T // 2                 # 512: matmul moving-dim / PSUM-bank size
F32 = mybir.dt.float32

# matmul input dtype: bf16 runs the PE at 1 cycle/row; fp32 at 4.
MM_DT = mybir.dt.float32 if os.environ.get("KERNEL_FP32") else mybir.dt.bfloat16


def build_nc() -> bass.Bass:
    nc = bacc.Bacc("TRN2")

    xs = nc.declare_dram_parameter("xs", [BLOC, T, C], F32, isOutput=False)
    w1t = nc.declare_dram_parameter("w1t", [CB, P, 2 * C], MM_DT, isOutput=False)
    w2s = nc.declare_dram_parameter("w2s", [K, OB, P, 2 * C], MM_DT, isOutput=False)
    w3t = nc.declare_dram_parameter("w3t", [CB, P, C], MM_DT, isOutput=False)
    b1 = nc.declare_dram_parameter("b1", [P, OB], F32, isOutput=False)
    b2 = nc.declare_dram_parameter("b2", [P, OB], F32, isOutput=False)
    b3 = nc.declare_dram_parameter("b3", [P, C], F32, isOutput=False)
    out = nc.declare_dram_parameter("out", [BLOC, T, C], F32, isOutput=True)

    with ExitStack() as ctx:
        tc = ctx.enter_context(tile.TileContext(nc))

        consts = ctx.enter_context(tc.tile_pool(name="consts", bufs=1))
        xin = ctx.enter_context(tc.tile_pool(name="xin", bufs=4))
        stats = ctx.enter_context(tc.tile_pool(name="stats", bufs=4))
        hNp = ctx.enter_context(tc.tile_pool(name="hNp", bufs=2))
        sigp = ctx.enter_context(tc.tile_pool(name="sigp", bufs=1))
        xbigp = ctx.enter_context(tc.tile_pool(name="xbigp", bufs=2))
        outp = ctx.enter_context(tc.tile_pool(name="outp", bufs=3))
        mm_psum = ctx.enter_context(tc.tile_pool(name="mm_psum", bufs=6, space="PSUM"))
        o_psum = ctx.enter_context(tc.tile_pool(name="o_psum", bufs=2, space="PSUM"))

        # ---- constants / weights (loaded once) ----
        epssb = consts.tile([P, 1], F32, tag="eps")
        nc.vector.memset(epssb, EPS_LN)
        b1sb = consts.tile([P, OB], F32, tag="b1")
        nc.sync.dma_start(out=b1sb, in_=b1[:])
        b2sb = consts.tile([P, OB], F32, tag="b2")
        nc.sync.dma_start(out=b2sb, in_=b2[:])
        b3sb = consts.tile([P, C], F32, tag="b3")
        nc.sync.dma_start(out=b3sb, in_=b3[:])
        w1sb = []
        for cb in range(CB):
            w = consts.tile([P, 2 * C], MM_DT, tag=f"w1_{cb}", name=f"w1_{cb}")
            nc.sync.dma_start(out=w, in_=w1t[cb])
            w1sb.append(w)
        w3sb = []
        for cb in range(CB):
            w = consts.tile([P, C], MM_DT, tag=f"w3_{cb}", name=f"w3_{cb}")
            nc.sync.dma_start(out=w, in_=w3t[cb])
            w3sb.append(w)
        w2sb = {}
        for k in range(K):
            for ib in range(OB):
                w = consts.tile([P, 2 * C], MM_DT, tag=f"w2_{k}_{ib}", name=f"w2_{k}_{ib}")
                nc.sync.dma_start(out=w, in_=w2s[k, ib])
                w2sb[(k, ib)] = w

        # Persistent activation tiles (reused across both batches).
        # h1 is the Swish output, zero-padded by 2 columns on each side so
        # the k=5 conv can slide its window without edge cases.
        h1 = []
        for ib in range(OB):
            t_ = consts.tile([P, T + 4], MM_DT, tag=f"h1_{ib}", name=f"h1_{ib}")
            nc.vector.memset(t_[:, 0:2], 0.0)
            nc.vector.memset(t_[:, T + 2 : T + 4], 0.0)
            h1.append(t_)
        hG = [consts.tile([P, T], MM_DT, tag=f"hG_{cb}", name=f"hG_{cb}") for cb in range(CB)]

        for b in range(BLOC):
            # ---- Phase A: LayerNorm ([tok, C] tiles) + PE transpose to
            # hN laid out [c-within-block, cb*T + t] ----
            # Per-token-block x loads into one fresh staging tile: LN of
            # block 0 starts as soon as its 256KB lands, not after 4MB.
            xbig = xbigp.tile([P, T // P, C], F32, tag="xbig")
            for tb in range(T // P):
                nc.sync.dma_start(
                    out=xbig[:, tb, :], in_=xs[b, tb * P : (tb + 1) * P, :]
                )
            hN = hNp.tile([P, CB * T], MM_DT, tag="hN")
            hN3 = hN[:, :].rearrange("p (c t) -> p c t", c=CB)
            for tb in range(T // P):
                xt = xbig[:, tb, :]
                st6 = stats.tile([P, 6], F32, tag="st6")
                nc.vector.bn_stats(out=st6, in_=xt)
                mv = stats.tile([P, 2], F32, tag="mv")
                nc.vector.bn_aggr(out=mv, in_=st6)
                rstd = stats.tile([P, 1], F32, tag="rstd")
                nc.scalar.activation(
                    out=rstd, in_=mv[:, 1:2],
                    func=mybir.ActivationFunctionType.Sqrt,
                    bias=epssb, scale=1.0,
                )
                nc.vector.reciprocal(out=rstd, in_=rstd)
                xn = xin.tile([P, C], MM_DT, tag="xn")
                nc.vector.tensor_scalar(
                    out=xn, in0=xt,
                    scalar1=mv[:, 0:1], scalar2=rstd,
                    op0=mybir.AluOpType.subtract, op1=mybir.AluOpType.mult,
                )
                # DMA xbar transpose (bf16) keeps the PE free for matmuls.
                for cb in range(CB):
                    nc.sync.dma_start_transpose(
                        out=hN3[:, cb, tb * P : (tb + 1) * P],
                        in_=xn[:, cb * P : (cb + 1) * P],
                    )

            # ---- Phase B: pointwise conv C->2C + Swish(psum + b1) ----
            for ob in range(OB):
                pA = mm_psum.tile([P, TH], F32, tag="mm")
                pB = mm_psum.tile([P, TH], F32, tag="mm")
                for cb in range(CB):
                    w = w1sb[cb][:, ob * P : (ob + 1) * P]
                    st, sp = cb == 0, cb == CB - 1
                    nc.tensor.matmul(pA, w, hN3[:, cb, 0:TH], start=st, stop=sp)
                    nc.tensor.matmul(pB, w, hN3[:, cb, TH:T], start=st, stop=sp)
                # Swish(z) = z * sigmoid(z), z = psum + b1
                for ph, psum, lo in ((0, pA, 2), (1, pB, 2 + TH)):
                    sg = sigp.tile([P, TH], MM_DT, tag=f"sw{ph}", name=f"sw{ph}")
                    nc.scalar.activation(
                        out=sg, in_=psum,
                        func=mybir.ActivationFunctionType.Sigmoid,
                        bias=b1sb[:, ob : ob + 1], scale=1.0,
                    )
                    z = sigp.tile([P, TH], MM_DT, tag=f"z{ph}", name=f"z{ph}")
                    nc.vector.tensor_scalar_add(
                        out=z, in0=psum, scalar1=b1sb[:, ob : ob + 1]
                    )
                    nc.vector.tensor_mul(
                        out=h1[ob][:, lo : lo + TH], in0=z, in1=sg
                    )

            # ---- Phase C: k=5 conv 2C->2C + GLU ----
            # Gate halves (ob 4..7) first so their sigmoids are ready when
            # the value halves (ob 0..3) drain.
            sig = {}
            for ob in [4, 5, 6, 7, 0, 1, 2, 3]:
                pA = mm_psum.tile([P, TH], F32, tag="mm")
                pB = mm_psum.tile([P, TH], F32, tag="mm")
                first = True
                for k in range(K):
                    for ib in range(OB):
                        w = w2sb[(k, ib)][:, ob * P : (ob + 1) * P]
                        last = (k == K - 1) and (ib == OB - 1)
                        nc.tensor.matmul(
                            pA, w, h1[ib][:, k : k + TH], start=first, stop=last
                        )
                        nc.tensor.matmul(
                            pB, w, h1[ib][:, TH + k : TH + k + TH],
                            start=first, stop=last,
                        )
                        first = False
                if ob >= 4:
                    j = ob - 4
                    sA = sigp.tile([P, TH], MM_DT, tag=f"sig{j}a", name=f"sig{j}a")
                    sB = sigp.tile([P, TH], MM_DT, tag=f"sig{j}b", name=f"sig{j}b")
                    nc.scalar.activation(
                        out=sA, in_=pA,
                        func=mybir.ActivationFunctionType.Sigmoid,
                        bias=b2sb[:, ob : ob + 1], scale=1.0,
                    )
                    nc.scalar.activation(
                        out=sB, in_=pB,
                        func=mybir.ActivationFunctionType.Sigmoid,
                        bias=b2sb[:, ob : ob + 1], scale=1.0,
                    )
                    sig[j] = (sA, sB)
                else:
                    j = ob
                    aA = sigp.tile([P, TH], MM_DT, tag=f"a{j}a", name=f"a{j}a")
                    aB = sigp.tile([P, TH], MM_DT, tag=f"a{j}b", name=f"a{j}b")
                    nc.vector.tensor_scalar_add(
                        out=aA, in0=pA, scalar1=b2sb[:, ob : ob + 1]
                    )
                    nc.vector.tensor_scalar_add(
                        out=aB, in0=pB, scalar1=b2sb[:, ob : ob + 1]
                    )
                    sA, sB = sig[j]
                    nc.vector.tensor_mul(out=hG[j][:, 0:TH], in0=aA, in1=sA)
                    nc.vector.tensor_mul(out=hG[j][:, TH:T], in0=aB, in1=sB)

            # ---- Phase D: pointwise conv C->C (+ BN fold) + bias ----
            # lhsT = activations so PSUM comes out [t, o] = DRAM layout.
            for tb in range(T // P):
                po = o_psum.tile([P, C], F32, tag="po")
                for cb in range(CB):
                    nc.tensor.matmul(
                        po,
                        hG[cb][:, tb * P : (tb + 1) * P],
                        w3sb[cb],
                        start=(cb == 0), stop=(cb == CB - 1),
                    )
                ot = outp.tile([P, C], F32, tag="ot")
                nc.vector.tensor_add(out=ot, in0=po, in1=b3sb)
                nc.gpsimd.dma_start(
                    out=out[b, tb * P : (tb + 1) * P, :], in_=ot
                )

    nc.compile()
    return nc


def prepare_inputs(x, ln_g, ln_b, w1, b1, w2, b2, bn_g, bn_b, bn_mean, bn_var, w3, b3):
    """Host-side folding + layout. Returns per-core input maps."""
    f = np.float32
    x = np.asarray(x, f)
    ln_g, ln_b = np.asarray(ln_g, f), np.asarray(ln_b, f)
    w1, b1 = np.asarray(w1, f), np.asarray(b1, f)
    w2, b2 = np.asarray(w2, f), np.asarray(b2, f)
    bn_g, bn_b = np.asarray(bn_g, f), np.asarray(bn_b, f)
    bn_mean, bn_var = np.asarray(bn_mean, f), np.asarray(bn_var, f)
    w3, b3 = np.asarray(w3, f), np.asarray(b3, f)

    # Fold LN affine into conv1, BN (eval) into conv3.
    w1f = w1 * ln_g[None, :]
    b1f = b1 + w1 @ ln_b
    s_bn = bn_g / np.sqrt(bn_var + EPS_BN)
    w3f = w3 * s_bn[None, :]
    b3f = b3 + w3 @ (bn_b - bn_mean * s_bn)

    mdt = mybir.dt.np(MM_DT)
    w1t = np.ascontiguousarray(w1f.T.reshape(CB, P, 2 * C)).astype(mdt)
    w2s = np.ascontiguousarray(w2.reshape(K, OB, P, 2 * C)).astype(mdt)
    w3t = np.ascontiguousarray(w3f.T.reshape(CB, P, C)).astype(mdt)
    b1d = np.ascontiguousarray(b1f.reshape(OB, P).T)
    b2d = np.ascontiguousarray(b2.reshape(OB, P).T)
    b3d = np.ascontiguousarray(np.broadcast_to(b3f, (P, C)))

    shared = {"w1t": w1t, "w2s": w2s, "w3t": w3t, "b1": b1d, "b2": b2d, "b3": b3d}
    in_maps = []
    for c in range(NCORES):
        m = dict(shared)
        m["xs"] = np.ascontiguousarray(x[c * BLOC : (c + 1) * BLOC])
        in_maps.append(m)
    return in_maps


_NC = None
LAST_RESULTS = None


def kernel(**inputs) -> np.ndarray:
    global _NC, LAST_RESULTS
    if _NC is None:
        _NC = build_nc()
    in_maps = prepare_inputs(**inputs)
    res = run_bass_kernel_spmd(_NC, in_maps, list(range(NCORES)))
    LAST_RESULTS = res
    return np.concatenate([r["out"] for r in res.results], axis=0)


# revision 18
# speedup vs baseline: 1.1324x; 1.1324x over previous
"""Trainium2 Bass kernel for the ConvModule problem.

Computes, for x (B=16, T=1024, C=512) fp32:
    h = LayerNorm_C(x) -> pw conv C->2C + Swish -> k=5 conv 2C->2C
      -> GLU -> BatchNorm(eval) -> pw conv C->C
Data-parallel over batch across 8 NeuronCores (2 batches/core, weights
replicated).  LN gamma/beta folded into w1/b1 and BN folded into w3/b3 on
the host, so the device only does: normalize, three matmul stages, Swish,
GLU.

Device data layout is [channel, time] (channels on partitions) for the
whole matmul chain; the final conv swaps matmul operands (activations as
the stationary lhsT) so its PSUM output lands directly in [time, channel]
DRAM layout.
"""

import os
from contextlib import ExitStack

import numpy as np

import concourse.bass as bass
import concourse.bacc as bacc
import concourse.tile as tile
from concourse import mybir
from concourse.masks import make_identity
from concourse.bass_utils import run_bass_kernel_spmd

B, T, C, K = 16, 1024, 512, 5
EPS_LN = 1e-5
EPS_BN = 1e-5
NCORES = 8
BLOC = B // NCORES          # batches per core
P = 128                     # SBUF partitions
CB = C // P                 # 4 channel blocks of the C dim
OB = (2 * C) // P           # 8 channel blocks of the 2C dim
TH = T // 2                 # 512: matmul moving-dim / PSUM-bank size
F32 = mybir.dt.float32

# matmul input dtype: bf16 runs the PE at 1 cycle/row; fp32 at 4.
MM_DT = mybir.dt.float32 if os.environ.get("KERNEL_FP32") else mybir.dt.bfloat16


def build_nc() -> bass.Bass:
    nc = bacc.Bacc("TRN2")

    xs = nc.declare_dram_parameter("xs", [BLOC, T, C], F32, isOutput=False)
    w1t = nc.declare_dram_parameter("w1t", [CB, P, 2 * C], MM_DT, isOutput=False)
    w2s = nc.declare_dram_parameter("w2s", [K, OB, P, 2 * C], MM_DT, isOutput=False)
    w3t = nc.declare_dram_parameter("w3t", [CB, P, C], MM_DT, isOutput=False)
    b1 = nc.declare_dram_parameter("b1", [P, OB], F32, isOutput=False)
    b2 = nc.declare_dram_parameter("b2", [P, OB], F32, isOutput=False)
    b3 = nc.declare_dram_parameter("b3", [P, C], F32, isOutput=False)
    out = nc.declare_dram_parameter("out", [BLOC, T, C], F32, isOutput=True)

    with ExitStack() as ctx:
        tc = ctx.enter_context(tile.TileContext(nc))

        consts = ctx.enter_context(tc.tile_pool(name="consts", bufs=1))
        xin = ctx.enter_context(tc.tile_pool(name="xin", bufs=4))
        stats = ctx.enter_context(tc.tile_pool(name="stats", bufs=4))
        hNp = ctx.enter_context(tc.tile_pool(name="hNp", bufs=2))
        sigp = ctx.enter_context(tc.tile_pool(name="sigp", bufs=1))
        xbigp = ctx.enter_context(tc.tile_pool(name="xbigp", bufs=2))
        outp = ctx.enter_context(tc.tile_pool(name="outp", bufs=3))
        tp_psum = ctx.enter_context(tc.tile_pool(name="tp_psum", bufs=2, space="PSUM"))
        mm_psum = ctx.enter_context(tc.tile_pool(name="mm_psum", bufs=4, space="PSUM"))
        o_psum = ctx.enter_context(tc.tile_pool(name="o_psum", bufs=2, space="PSUM"))

        # ---- constants / weights (loaded once) ----
        ident = consts.tile([P, P], MM_DT, tag="ident")
        make_identity(nc, ident)
        epssb = consts.tile([P, 1], F32, tag="eps")
        nc.vector.memset(epssb, EPS_LN)
        b1sb = consts.tile([P, OB], F32, tag="b1")
        nc.sync.dma_start(out=b1sb, in_=b1[:])
        b2sb = consts.tile([P, OB], F32, tag="b2")
        nc.sync.dma_start(out=b2sb, in_=b2[:])
        b3sb = consts.tile([P, C], F32, tag="b3")
        nc.sync.dma_start(out=b3sb, in_=b3[:])
        w1sb = []
        for cb in range(CB):
            w = consts.tile([P, 2 * C], MM_DT, tag=f"w1_{cb}", name=f"w1_{cb}")
            nc.sync.dma_start(out=w, in_=w1t[cb])
            w1sb.append(w)
        w3sb = []
        for cb in range(CB):
            w = consts.tile([P, C], MM_DT, tag=f"w3_{cb}", name=f"w3_{cb}")
            nc.sync.dma_start(out=w, in_=w3t[cb])
            w3sb.append(w)
        w2sb = {}
        for k in range(K):
            for ib in range(OB):
                w = consts.tile([P, 2 * C], MM_DT, tag=f"w2_{k}_{ib}", name=f"w2_{k}_{ib}")
                nc.sync.dma_start(out=w, in_=w2s[k, ib])
                w2sb[(k, ib)] = w

        # Persistent activation tiles (reused across both batches).
        # h1 is the Swish output, zero-padded by 2 columns on each side so
        # the k=5 conv can slide its window without edge cases.
        h1 = []
        for ib in range(OB):
            t_ = consts.tile([P, T + 4], MM_DT, tag=f"h1_{ib}", name=f"h1_{ib}")
            nc.vector.memset(t_[:, 0:2], 0.0)
            nc.vector.memset(t_[:, T + 2 : T + 4], 0.0)
            h1.append(t_)
        hG = [consts.tile([P, T], MM_DT, tag=f"hG_{cb}", name=f"hG_{cb}") for cb in range(CB)]

        for b in range(BLOC):
            # ---- Phase A: LayerNorm ([tok, C] tiles) + PE transpose to
            # hN laid out [c-within-block, cb*T + t] ----
            # Per-token-block x loads into one fresh staging tile: LN of
            # block 0 starts as soon as its 256KB lands, not after 4MB.
            xbig = xbigp.tile([P, T // P, C], F32, tag="xbig")
            for tb in range(T // P):
                nc.sync.dma_start(
                    out=xbig[:, tb, :], in_=xs[b, tb * P : (tb + 1) * P, :]
                )
            hN = hNp.tile([P, CB * T], MM_DT, tag="hN")
            hN3 = hN[:, :].rearrange("p (c t) -> p c t", c=CB)
            for tb in range(T // P):
                xt = xbig[:, tb, :]
                st6 = stats.tile([P, 6], F32, tag="st6")
                nc.vector.bn_stats(out=st6, in_=xt)
                mv = stats.tile([P, 2], F32, tag="mv")
                nc.vector.bn_aggr(out=mv, in_=st6)
                rstd = stats.tile([P, 1], F32, tag="rstd")
                nc.scalar.activation(
                    out=rstd, in_=mv[:, 1:2],
                    func=mybir.ActivationFunctionType.Sqrt,
                    bias=epssb, scale=1.0,
                )
                nc.vector.reciprocal(out=rstd, in_=rstd)
                xn = xin.tile([P, C], MM_DT, tag="xn")
                nc.vector.tensor_scalar(
                    out=xn, in0=xt,
                    scalar1=mv[:, 0:1], scalar2=rstd,
                    op0=mybir.AluOpType.subtract, op1=mybir.AluOpType.mult,
                )
                ps = tp_psum.tile([P, CB * P], MM_DT, tag="tp")
                for cb in range(CB):
                    nc.tensor.transpose(
                        ps[:, cb * P : (cb + 1) * P],
                        xn[:, cb * P : (cb + 1) * P],
                        ident,
                    )
                nc.scalar.copy(
                    out=hN3[:, :, tb * P : (tb + 1) * P],
                    in_=ps[:, :].rearrange("p (c i) -> p c i", c=CB),
                )

            # ---- Phase B: pointwise conv C->2C + Swish(psum + b1) ----
            for ob in range(OB):
                pA = mm_psum.tile([P, TH], F32, tag="mm")
                pB = mm_psum.tile([P, TH], F32, tag="mm")
                for cb in range(CB):
                    w = w1sb[cb][:, ob * P : (ob + 1) * P]
                    st, sp = cb == 0, cb == CB - 1
                    nc.tensor.matmul(pA, w, hN3[:, cb, 0:TH], start=st, stop=sp)
                    nc.tensor.matmul(pB, w, hN3[:, cb, TH:T], start=st, stop=sp)
                # Swish(z) = z * sigmoid(z), z = psum + b1
                for ph, psum, lo in ((0, pA, 2), (1, pB, 2 + TH)):
                    sg = sigp.tile([P, TH], MM_DT, tag=f"sw{ph}", name=f"sw{ph}")
                    nc.scalar.activation(
                        out=sg, in_=psum,
                        func=mybir.ActivationFunctionType.Sigmoid,
                        bias=b1sb[:, ob : ob + 1], scale=1.0,
                    )
                    z = sigp.tile([P, TH], MM_DT, tag=f"z{ph}", name=f"z{ph}")
                    nc.vector.tensor_scalar_add(
                        out=z, in0=psum, scalar1=b1sb[:, ob : ob + 1]
                    )
                    nc.vector.tensor_mul(
                        out=h1[ob][:, lo : lo + TH], in0=z, in1=sg
                    )

            # ---- Phase C: k=5 conv 2C->2C + GLU ----
            # Gate halves (ob 4..7) first so their sigmoids are ready when
            # the value halves (ob 0..3) drain.
            sig = {}
            for ob in [4, 5, 6, 7, 0, 1, 2, 3]:
                pA = mm_psum.tile([P, TH], F32, tag="mm")
                pB = mm_psum.tile([P, TH], F32, tag="mm")
                first = True
                for k in range(K):
                    for ib in range(OB):
                        w = w2sb[(k, ib)][:, ob * P : (ob + 1) * P]
                        last = (k == K - 1) and (ib == OB - 1)
                        nc.tensor.matmul(
                            pA, w, h1[ib][:, k : k + TH], start=first, stop=last
                        )
                        nc.tensor.matmul(
                            pB, w, h1[ib][:, TH + k : TH + k + TH],
                            start=first, stop=last,
                        )
                        first = False
                if ob >= 4:
                    j = ob - 4
                    sA = sigp.tile([P, TH], MM_DT, tag=f"sig{j}a", name=f"sig{j}a")
                    sB = sigp.tile([P, TH], MM_DT, tag=f"sig{j}b", name=f"sig{j}b")
                    nc.scalar.activation(
                        out=sA, in_=pA,
                        func=mybir.ActivationFunctionType.Sigmoid,
                        bias=b2sb[:, ob : ob + 1], scale=1.0,
                    )
                    nc.scalar.activation(
                        out=sB, in_=pB,
                        func=mybir.ActivationFunctionType.Sigmoid,
                        bias=b2sb[:, ob : ob + 1], scale=1.0,
                    )
                    sig[j] = (sA, sB)
                else:
                    j = ob
                    aA = sigp.tile([P, TH], MM_DT, tag=f"a{j}a", name=f"a{j}a")
                    aB = sigp.tile([P, TH], MM_DT, tag=f"a{j}b", name=f"a{j}b")
                    nc.vector.tensor_scalar_add(
                        out=aA, in0=pA, scalar1=b2sb[:, ob : ob + 1]
                    )
                    nc.vector.tensor_scalar_add(
                        out=aB, in0=pB, scalar1=b2sb[:, ob : ob + 1]
                    )
                    sA, sB = sig[j]
                    nc.vector.tensor_mul(out=hG[j][:, 0:TH], in0=aA, in1=sA)
                    nc.vector.tensor_mul(out=hG[j][:, TH:T], in0=aB, in1=sB)

            # ---- Phase D: pointwise conv C->C (+ BN fold) + bias ----
            # lhsT = activations so PSUM comes out [t, o] = DRAM layout.
            for tb in range(T // P):
                po = o_psum.tile([P, C], F32, tag="po")
                for cb in range(CB):
                    nc.tensor.matmul(
                        po,
                        hG[cb][:, tb * P : (tb + 1) * P],
                        w3sb[cb],
                        start=(cb == 0), stop=(cb == CB - 1),
                    )
                ot = outp.tile([P, C], F32, tag="ot")
                nc.vector.tensor_add(out=ot, in0=po, in1=b3sb)
                nc.gpsimd.dma_start(
                    out=out[b, tb * P : (tb + 1) * P, :], in_=ot
                )

    nc.compile()
    return nc


def prepare_inputs(x, ln_g, ln_b, w1, b1, w2, b2, bn_g, bn_b, bn_mean, bn_var, w3, b3):
    """Host-side folding + layout. Returns per-core input maps."""
    f = np.float32
    x = np.asarray(x, f)
    ln_g, ln_b = np.asarray(ln_g, f), np.asarray(ln_b, f)
    w1, b1 = np.asarray(w1, f), np.asarray(b1, f)
    w2, b2 = np.asarray(w2, f), np.asarray(b2, f)
    bn_g, bn_b = np.asarray(bn_g, f), np.asarray(bn_b, f)
    bn_mean, bn_var = np.asarray(bn_mean, f), np.asarray(bn_var, f)
    w3, b3 = np.asarray(w3, f), np.asarray(b3, f)

    # Fold LN affine into conv1, BN (eval) into conv3.
    w1f = w1 * ln_g[None, :]
    b1f = b1 + w1 @ ln_b
    s_bn = bn_g / np.sqrt(bn_var + EPS_BN)
    w3f = w3 * s_bn[None, :]
    b3f = b3 + w3 @ (bn_b - bn_mean * s_bn)

    mdt = mybir.dt.np(MM_DT)
    w1t = np.ascontiguousarray(w1f.T.reshape(CB, P, 2 * C)).astype(mdt)
    w2s = np.ascontiguousarray(w2.reshape(K, OB, P, 2 * C)).astype(mdt)
    w3t = np.ascontiguousarray(w3f.T.reshape(CB, P, C)).astype(mdt)
    b1d = np.ascontiguousarray(b1f.reshape(OB, P).T)
    b2d = np.ascontiguousarray(b2.reshape(OB, P).T)
    b3d = np.ascontiguousarray(np.broadcast_to(b3f, (P, C)))

    shared = {"w1t": w1t, "w2s": w2s, "w3t": w3t, "b1": b1d, "b2": b2d, "b3": b3d}
    in_maps = []
    for c in range(NCORES):
        m = dict(shared)
        m["xs"] = np.ascontiguousarray(x[c * BLOC : (c + 1) * BLOC])
        in_maps.append(m)
    return in_maps


_NC = None
LAST_RESULTS = None


def kernel(**inputs) -> np.ndarray:
    global _NC, LAST_RESULTS
    if _NC is None:
        _NC = build_nc()
    in_maps = prepare_inputs(**inputs)
    res = run_bass_kernel_spmd(_NC, in_maps, list(range(NCORES)))
    LAST_RESULTS = res
    return np.concatenate([r["out"] for r in res.results], axis=0)
